# revision 15
# baseline (speedup 1.0000x reference)
"""Trainium2 Bass kernel for CosineSimilarityLoss.

Reference math (see problem):
    x1 = a[labels]; x2 = b[labels]          # gather rows, [N, D]
    ip = sum(x1*x2, -1); w1 = ||x1||; w2 = ||x2||
    cos = ip / max(w1*w2, 1e-8)
    mask = cos >= 0.1
    out = sum(cos[mask]) / max(count(mask), 1)

Sharding: rows of a/b are sharded across 8 cores (12500 rows each). The
host partitions `labels` by owning shard; each core gathers only its local
rows via indirect DMA, computes the masked partial sum and count, and the
host combines the 8 (sum, count) pairs.

Per-core device kernel:
  - inputs:  ab  [12500, 1024] f32 (concat a_shard | b_shard: one gather
             descriptor fetches both rows)
             idx [128, NT] i32 (grid (p,c) holds label slot c*128+p; pads
             point at row 0)
             w   [128, NT] f32 (1.0 real label, 0.0 pad)
  - output:  out [1, 2] f32 = (masked partial sum, partial count)

Engine split per super-chunk of up to 4 label-columns:
  - GpSimd: one indirect DMA per column (128 rows x 4KB) into slices of a
    shared super-tile (Q7 descriptor emission ~10ns/desc is a hard serial
    floor, ~20us for 2176 rows — same for dma_gather, which also needs a
    5us ucode library load, so plain indirect DMA wins)
  - DVE:    batched product a*b + free-axis reduce -> ip over the super-
    tile; plus |b|^2 for one super-chunk (load balance)
  - ACT:    Square activation with fused accumulation for the norms
"""

import math
import sys

import numpy as np

if "/opt/trn_rl_repo" not in sys.path:
    sys.path.append("/opt/trn_rl_repo")

V = 100000
D = 512
N_CORES = 8
R = V // N_CORES
P = 128
EPS = 1e-8
MIN_THRESH = 0.1
TG = 4  # label-columns per super-chunk

_CACHE: dict = {}


def _super_chunks(nt: int):
    """Column ranges per super-chunk; first one small to start the compute
    pipeline early."""
    out = []
    c = 0
    first = True
    while c < nt:
        tcs = min(1 if first else TG, nt - c)
        out.append((c, tcs))
        c += tcs
        first = False
    return out


def _build_program(nt: int, rows: int = R, d: int = D):
    import concourse.bacc as bacc
    import concourse.bass as bass
    import concourse.mybir as mybir
    import concourse.tile as tile

    f32 = mybir.dt.float32
    Alu = mybir.AluOpType
    Act = mybir.ActivationFunctionType

    nc = bacc.Bacc(
        "TRN2",
        target_bir_lowering=False,
        debug=False,
        enable_asserts=False,
        num_devices=N_CORES,
    )
    ab = nc.dram_tensor("ab", [rows, 2 * d], f32, kind="ExternalInput").ap()
    idx = nc.dram_tensor("idx", [P, nt], mybir.dt.int32, kind="ExternalInput").ap()
    wv = nc.dram_tensor("w", [P, nt], f32, kind="ExternalInput").ap()
    out = nc.dram_tensor("out", [1, 2], f32, kind="ExternalOutput").ap()

    with tile.TileContext(nc) as tc:
        with (
            tc.tile_pool(name="persist", bufs=1) as persist,
            tc.tile_pool(name="gather", bufs=6) as gpool,
            tc.tile_pool(name="scrv", bufs=2) as scrv,
            tc.tile_pool(name="dumm", bufs=1) as dummp,
            tc.tile_pool(name="tail", bufs=1) as tailp,
            tc.tile_pool(name="psum", bufs=1, space="PSUM") as psump,
        ):
            idx_sb = persist.tile([P, nt], mybir.dt.int32)
            w_sb = persist.tile([P, nt], f32)
            ip_sb = persist.tile([P, nt], f32)
            n1_sb = persist.tile([P, nt], f32)
            n2_sb = persist.tile([P, nt], f32)
            # split the index load so the first gather only waits on col 0
            nc.sync.dma_start(out=idx_sb[:, 0:1], in_=idx[:, 0:1])
            if nt > 1:
                nc.sync.dma_start(out=idx_sb[:, 1:nt], in_=idx[:, 1:nt])
            nc.sync.dma_start(out=w_sb[:], in_=wv)

            # write-only sink for ACT Square ops (their real output is the
            # fused accumulator); same-engine program order makes reuse safe
            act_dummy = dummp.tile([P, 1], f32)
            # pre-warm the Sqrt activation table off the critical path (the
            # first use would otherwise pay a ~1.3us inline table load +
            # engine drain right inside the tail chain)
            warm = dummp.tile([P, 1], f32)
            nc.vector.memset(warm[:], 1.0)
            nc.scalar.activation(act_dummy[:], warm[:], Act.Sqrt)

            sc_list = _super_chunks(nt)
            n_sc = len(sc_list)
            for sc_i, (c0, tcs) in enumerate(sc_list):
                g = gpool.tile([P, TG, 2 * d], f32, tag="g")
                for t in range(tcs):
                    nc.gpsimd.indirect_dma_start(
                        out=g[:, t, :],
                        out_offset=None,
                        in_=ab,
                        in_offset=bass.IndirectOffsetOnAxis(
                            ap=idx_sb[:, c0 + t : c0 + t + 1], axis=0
                        ),
                    )
                av = g[:, 0:tcs, 0:d]
                bv = g[:, 0:tcs, d : 2 * d]
                # ip = reduce(a*b), batched over tcs columns (fused DVE accum
                # ops crash/fail to compile on this stack, so two ops; Pool
                # products serialize into the tail after emission — net loss)
                so = scrv.tile([P, TG, d], f32, tag="scrv")
                nc.vector.tensor_tensor(
                    out=so[:, 0:tcs, :], in0=av, in1=bv, op=Alu.mult
                )
                nc.vector.tensor_reduce(
                    ip_sb[:, c0 : c0 + tcs],
                    so[:, 0:tcs, :],
                    axis=mybir.AxisListType.X,
                    op=Alu.add,
                )
                # norms: ACT Square+accum per column; one super-chunk's n2
                # goes to DVE to balance engine load
                n2_on_dve = sc_i == 1 and tcs > 1
                if n2_on_dve:
                    so2 = scrv.tile([P, TG, d], f32, tag="scrv")
                    nc.vector.tensor_tensor(
                        out=so2[:, 0:tcs, :], in0=bv, in1=bv, op=Alu.mult
                    )
                    nc.vector.tensor_reduce(
                        n2_sb[:, c0 : c0 + tcs],
                        so2[:, 0:tcs, :],
                        axis=mybir.AxisListType.X,
                        op=Alu.add,
                    )
                for t in range(tcs):
                    c = c0 + t
                    nc.scalar.activation(
                        act_dummy[:].broadcast_to([P, d]),
                        g[:, t, 0:d],
                        Act.Square,
                        accum_out=n1_sb[:, c : c + 1],
                    )
                    if not n2_on_dve:
                        nc.scalar.activation(
                            act_dummy[:].broadcast_to([P, d]),
                            g[:, t, d : 2 * d],
                            Act.Square,
                            accum_out=n2_sb[:, c : c + 1],
                        )

            # tail: cos = ip / max(sqrt(n1*n2), eps); masked sum + count
            nn = tailp.tile([P, nt], f32)
            nc.vector.tensor_tensor(out=nn[:], in0=n1_sb[:], in1=n2_sb[:], op=Alu.mult)
            den0 = tailp.tile([P, nt], f32)
            nc.scalar.activation(den0[:], nn[:], Act.Sqrt)
            den = tailp.tile([P, nt], f32)
            nc.vector.tensor_scalar(
                out=den[:], in0=den0[:], scalar1=EPS, scalar2=None, op0=Alu.max
            )
            rec = tailp.tile([P, nt], f32)
            nc.vector.reciprocal(rec[:], den[:])
            cosv = tailp.tile([P, nt], f32)
            nc.vector.tensor_tensor(out=cosv[:], in0=ip_sb[:], in1=rec[:], op=Alu.mult)
            mk = tailp.tile([P, nt], f32)
            nc.vector.tensor_scalar(
                out=mk[:], in0=cosv[:], scalar1=MIN_THRESH, scalar2=None, op0=Alu.is_ge
            )
            mw = tailp.tile([P, nt], f32)
            nc.vector.tensor_tensor(out=mw[:], in0=mk[:], in1=w_sb[:], op=Alu.mult)
            mc = tailp.tile([P, nt], f32)
            nc.vector.tensor_tensor(out=mc[:], in0=cosv[:], in1=mw[:], op=Alu.mult)

            sc = tailp.tile([P, 2], f32)
            nc.vector.tensor_reduce(
                sc[:, 0:1], mc[:], axis=mybir.AxisListType.X, op=Alu.add
            )
            nc.vector.tensor_reduce(
                sc[:, 1:2], mw[:], axis=mybir.AxisListType.X, op=Alu.add
            )
            ones = tailp.tile([P, 1], f32)
            nc.vector.memset(ones[:], 1.0)
            ps = psump.tile([1, 2], f32, space="PSUM")
            nc.tensor.matmul(ps[:], lhsT=ones[:], rhs=sc[:], start=True, stop=True)
            osb = tailp.tile([1, 2], f32)
            nc.vector.tensor_copy(out=osb[:], in_=ps[:])
            nc.sync.dma_start(out=out, in_=osb[:])

    nc.compile()
    return nc


def _get_program(nt: int):
    key = ("prog", nt)
    if key not in _CACHE:
        _CACHE[key] = _build_program(nt)
    return _CACHE[key]


def _shard_host(a, b, labels):
    """Partition labels by owning row-shard; build per-core inputs."""
    a = np.ascontiguousarray(np.asarray(a, dtype=np.float32))
    b = np.ascontiguousarray(np.asarray(b, dtype=np.float32))
    lab = np.asarray(labels).astype(np.int64).ravel()

    # dedupe duplicate labels per shard: gather each distinct row once and
    # weight its (identical) cosine by the multiplicity — same value and
    # count as the reference, ~8-12% less DMA/compute
    locs = []
    for dcore in range(N_CORES):
        lo = dcore * R
        sel = lab[(lab >= lo) & (lab < lo + R)] - lo
        uniq, cnts = np.unique(sel, return_counts=True)
        locs.append((uniq.astype(np.int32), cnts.astype(np.float32)))
    kmax = max(len(u) for u, _ in locs)
    nt = max(1, math.ceil(kmax / P))
    kpad = nt * P

    in_maps = []
    for dcore in range(N_CORES):
        lo = dcore * R
        uniq, cnts = locs[dcore]
        flat = np.zeros(kpad, dtype=np.int32)
        flat[: len(uniq)] = uniq
        w_flat = np.zeros(kpad, dtype=np.float32)
        w_flat[: len(uniq)] = cnts
        # grid position (p, c) holds flat slot c*128+p
        idx2d = np.ascontiguousarray(flat.reshape(nt, P).T)
        w2d = np.ascontiguousarray(w_flat.reshape(nt, P).T)
        ab = np.concatenate([a[lo : lo + R], b[lo : lo + R]], axis=1)
        in_maps.append(
            {"ab": np.ascontiguousarray(ab), "idx": idx2d, "w": w2d}
        )
    return nt, in_maps


def run_sharded(a, b, labels, **run_kwargs):
    """Shard, run on 8 cores, return (result_scalar, BassKernelResults)."""
    from concourse.bass_utils import run_bass_kernel_spmd

    nt, in_maps = _shard_host(a, b, labels)
    nc = _get_program(nt)
    res = run_bass_kernel_spmd(nc, in_maps, list(range(N_CORES)), **run_kwargs)
    partials = np.stack([r["out"][0] for r in res.results])  # [8, 2]
    total = np.float32(partials[:, 0].astype(np.float64).sum())
    cnt = max(int(round(float(partials[:, 1].sum()))), 1)
    value = np.asarray(np.float32(total) / np.float32(cnt))
    return value, res


def kernel(a, b, labels):
    value, _ = run_sharded(a, b, labels)
    return value


# revision 20
# speedup vs baseline: 1.0576x; 1.0576x over previous
"""Trainium2 Bass kernel for CosineSimilarityLoss.

Reference math (see problem):
    x1 = a[labels]; x2 = b[labels]          # gather rows, [N, D]
    ip = sum(x1*x2, -1); w1 = ||x1||; w2 = ||x2||
    cos = ip / max(w1*w2, 1e-8)
    mask = cos >= 0.1
    out = sum(cos[mask]) / max(count(mask), 1)

Sharding: rows of a/b are sharded across 8 cores (12500 rows each). The
host partitions `labels` by owning shard; each core gathers only its local
rows via indirect DMA, computes the masked partial sum and count, and the
host combines the 8 (sum, count) pairs.

Per-core device kernel:
  - inputs:  ab  [12500, 1024] f32 (concat a_shard | b_shard: one gather
             descriptor fetches both rows)
             idx [128, NT] i32 (grid (p,c) holds label slot c*128+p; pads
             point at row 0)
             w   [128, NT] f32 (1.0 real label, 0.0 pad)
  - output:  out [1, 2] f32 = (masked partial sum, partial count)

Engine split per super-chunk of up to 4 label-columns:
  - GpSimd: one indirect DMA per column (128 rows x 4KB) into slices of a
    shared super-tile (Q7 descriptor emission ~10ns/desc is a hard serial
    floor, ~20us for 2176 rows — same for dma_gather, which also needs a
    5us ucode library load, so plain indirect DMA wins)
  - DVE:    batched product a*b + free-axis reduce -> ip over the super-
    tile; plus |b|^2 for one super-chunk (load balance)
  - ACT:    Square activation with fused accumulation for the norms
"""

import math
import sys

import numpy as np

if "/opt/trn_rl_repo" not in sys.path:
    sys.path.append("/opt/trn_rl_repo")


def _ensure_axon_hooks_stub():
    """concourse's axon trace path imports antenv.axon_hooks, which some
    agent images lack; a BASS_TRACE=1 environment would then crash the run.
    Provide a stub that degrades tracing gracefully."""
    try:
        import antenv.axon_hooks  # noqa: F401
        return
    except Exception:
        pass
    try:
        import types

        import antenv

        mod = types.ModuleType("antenv.axon_hooks")
        mod.get_axon_ntff_profile_hook = lambda: None
        mod.set_axon_ntff_profile_hook = lambda h: None
        antenv.axon_hooks = mod
        sys.modules["antenv.axon_hooks"] = mod
    except Exception:
        pass


_ensure_axon_hooks_stub()

V = 100000
D = 512
N_CORES = 8
R = V // N_CORES
P = 128
EPS = 1e-8
MIN_THRESH = 0.1
TG = 4  # label-columns per super-chunk

_CACHE: dict = {}


def _super_chunks(nt: int):
    """Column ranges per super-chunk; first one small to start the compute
    pipeline early."""
    out = []
    c = 0
    first = True
    while c < nt:
        tcs = min(1 if first else TG, nt - c)
        out.append((c, tcs))
        c += tcs
        first = False
    return out


def _build_program(nt: int, rows: int = R, d: int = D):
    import concourse.bacc as bacc
    import concourse.bass as bass
    import concourse.mybir as mybir
    import concourse.tile as tile

    f32 = mybir.dt.float32
    Alu = mybir.AluOpType
    Act = mybir.ActivationFunctionType

    nc = bacc.Bacc(
        "TRN2",
        target_bir_lowering=False,
        debug=False,
        enable_asserts=False,
        num_devices=N_CORES,
    )
    ab = nc.dram_tensor("ab", [rows, 2 * d], f32, kind="ExternalInput").ap()
    idx = nc.dram_tensor("idx", [P, nt], mybir.dt.int32, kind="ExternalInput").ap()
    wv = nc.dram_tensor("w", [P, nt], f32, kind="ExternalInput").ap()
    out = nc.dram_tensor("out", [1, 2], f32, kind="ExternalOutput").ap()

    with tile.TileContext(nc) as tc:
        with (
            tc.tile_pool(name="persist", bufs=1) as persist,
            tc.tile_pool(name="gather", bufs=6) as gpool,
            tc.tile_pool(name="scrv", bufs=2) as scrv,
            tc.tile_pool(name="dumm", bufs=1) as dummp,
            tc.tile_pool(name="tail", bufs=1) as tailp,
            tc.tile_pool(name="psum", bufs=1, space="PSUM") as psump,
        ):
            idx_sb = persist.tile([P, nt], mybir.dt.int32)
            w_sb = persist.tile([P, nt], f32)
            ip_sb = persist.tile([P, nt], f32)
            n1_sb = persist.tile([P, nt], f32)
            n2_sb = persist.tile([P, nt], f32)
            # split the index load so the first gather only waits on col 0
            nc.sync.dma_start(out=idx_sb[:, 0:1], in_=idx[:, 0:1])
            if nt > 1:
                nc.sync.dma_start(out=idx_sb[:, 1:nt], in_=idx[:, 1:nt])
            nc.sync.dma_start(out=w_sb[:], in_=wv)

            # write-only sink for ACT Square ops (their real output is the
            # fused accumulator); same-engine program order makes reuse safe
            act_dummy = dummp.tile([P, 1], f32)
            # pre-warm the Sqrt activation table off the critical path (the
            # first use would otherwise pay a ~1.3us inline table load +
            # engine drain right inside the tail chain)
            warm = dummp.tile([P, 1], f32)
            nc.vector.memset(warm[:], 1.0)
            nc.scalar.activation(act_dummy[:], warm[:], Act.Sqrt)

            sc_list = _super_chunks(nt)
            n_sc = len(sc_list)
            for sc_i, (c0, tcs) in enumerate(sc_list):
                g = gpool.tile([P, TG, 2 * d], f32, tag="g")
                for t in range(tcs):
                    nc.gpsimd.indirect_dma_start(
                        out=g[:, t, :],
                        out_offset=None,
                        in_=ab,
                        in_offset=bass.IndirectOffsetOnAxis(
                            ap=idx_sb[:, c0 + t : c0 + t + 1], axis=0
                        ),
                    )
                av = g[:, 0:tcs, 0:d]
                bv = g[:, 0:tcs, d : 2 * d]
                # ip = reduce(a*b), batched over tcs columns (fused DVE accum
                # ops crash/fail to compile on this stack, so two ops; Pool
                # products serialize into the tail after emission — net loss)
                so = scrv.tile([P, TG, d], f32, tag="scrv")
                nc.vector.tensor_tensor(
                    out=so[:, 0:tcs, :], in0=av, in1=bv, op=Alu.mult
                )
                nc.vector.tensor_reduce(
                    ip_sb[:, c0 : c0 + tcs],
                    so[:, 0:tcs, :],
                    axis=mybir.AxisListType.X,
                    op=Alu.add,
                )
                # norms: ACT Square+accum per column, except ~6-7 columns of
                # n2 on DVE to equalize engine load (ACT ~1.0us/col vs DVE
                # ~1.12us/col for a square+reduce)
                n2_dve_cols = tcs if sc_i == 1 else 0
                if n2_dve_cols:
                    so2 = scrv.tile([P, TG, d], f32, tag="scrv")
                    bvd = g[:, 0:n2_dve_cols, d : 2 * d]
                    nc.vector.tensor_tensor(
                        out=so2[:, 0:n2_dve_cols, :], in0=bvd, in1=bvd, op=Alu.mult
                    )
                    nc.vector.tensor_reduce(
                        n2_sb[:, c0 : c0 + n2_dve_cols],
                        so2[:, 0:n2_dve_cols, :],
                        axis=mybir.AxisListType.X,
                        op=Alu.add,
                    )
                for t in range(tcs):
                    c = c0 + t
                    nc.scalar.activation(
                        act_dummy[:].broadcast_to([P, d]),
                        g[:, t, 0:d],
                        Act.Square,
                        accum_out=n1_sb[:, c : c + 1],
                    )
                    if t >= n2_dve_cols:
                        nc.scalar.activation(
                            act_dummy[:].broadcast_to([P, d]),
                            g[:, t, d : 2 * d],
                            Act.Square,
                            accum_out=n2_sb[:, c : c + 1],
                        )

            # tail: cos = ip / max(sqrt(n1*n2), eps); masked sum + count
            nn = tailp.tile([P, nt], f32)
            nc.vector.tensor_tensor(out=nn[:], in0=n1_sb[:], in1=n2_sb[:], op=Alu.mult)
            den0 = tailp.tile([P, nt], f32)
            nc.scalar.activation(den0[:], nn[:], Act.Sqrt)
            den = tailp.tile([P, nt], f32)
            nc.vector.tensor_scalar(
                out=den[:], in0=den0[:], scalar1=EPS, scalar2=None, op0=Alu.max
            )
            rec = tailp.tile([P, nt], f32)
            nc.vector.reciprocal(rec[:], den[:])
            cosv = tailp.tile([P, nt], f32)
            nc.vector.tensor_tensor(out=cosv[:], in0=ip_sb[:], in1=rec[:], op=Alu.mult)
            mk = tailp.tile([P, nt], f32)
            nc.vector.tensor_scalar(
                out=mk[:], in0=cosv[:], scalar1=MIN_THRESH, scalar2=None, op0=Alu.is_ge
            )
            mw = tailp.tile([P, nt], f32)
            nc.vector.tensor_tensor(out=mw[:], in0=mk[:], in1=w_sb[:], op=Alu.mult)
            mc = tailp.tile([P, nt], f32)
            nc.vector.tensor_tensor(out=mc[:], in0=cosv[:], in1=mw[:], op=Alu.mult)

            sc = tailp.tile([P, 2], f32)
            nc.vector.tensor_reduce(
                sc[:, 0:1], mc[:], axis=mybir.AxisListType.X, op=Alu.add
            )
            nc.vector.tensor_reduce(
                sc[:, 1:2], mw[:], axis=mybir.AxisListType.X, op=Alu.add
            )
            ones = tailp.tile([P, 1], f32)
            nc.vector.memset(ones[:], 1.0)
            ps = psump.tile([1, 2], f32, space="PSUM")
            nc.tensor.matmul(ps[:], lhsT=ones[:], rhs=sc[:], start=True, stop=True)
            osb = tailp.tile([1, 2], f32)
            nc.vector.tensor_copy(out=osb[:], in_=ps[:])
            nc.sync.dma_start(out=out, in_=osb[:])

    nc.compile()
    return nc


def _get_program(nt: int):
    key = ("prog", nt)
    if key not in _CACHE:
        _CACHE[key] = _build_program(nt)
    return _CACHE[key]


def _shard_host(a, b, labels):
    """Partition labels by owning row-shard; build per-core inputs."""
    a = np.ascontiguousarray(np.asarray(a, dtype=np.float32))
    b = np.ascontiguousarray(np.asarray(b, dtype=np.float32))
    lab = np.asarray(labels).astype(np.int64).ravel()

    # dedupe duplicate labels per shard: gather each distinct row once and
    # weight its (identical) cosine by the multiplicity — same value and
    # count as the reference, ~8-12% less DMA/compute
    locs = []
    for dcore in range(N_CORES):
        lo = dcore * R
        sel = lab[(lab >= lo) & (lab < lo + R)] - lo
        uniq, cnts = np.unique(sel, return_counts=True)
        locs.append((uniq.astype(np.int32), cnts.astype(np.float32)))
    kmax = max(len(u) for u, _ in locs)
    nt = max(1, math.ceil(kmax / P))
    kpad = nt * P

    in_maps = []
    for dcore in range(N_CORES):
        lo = dcore * R
        uniq, cnts = locs[dcore]
        flat = np.zeros(kpad, dtype=np.int32)
        flat[: len(uniq)] = uniq
        w_flat = np.zeros(kpad, dtype=np.float32)
        w_flat[: len(uniq)] = cnts
        # grid position (p, c) holds flat slot c*128+p
        idx2d = np.ascontiguousarray(flat.reshape(nt, P).T)
        w2d = np.ascontiguousarray(w_flat.reshape(nt, P).T)
        ab = np.concatenate([a[lo : lo + R], b[lo : lo + R]], axis=1)
        in_maps.append(
            {"ab": np.ascontiguousarray(ab), "idx": idx2d, "w": w2d}
        )
    return nt, in_maps


def run_sharded(a, b, labels, **run_kwargs):
    """Shard, run on 8 cores, return (result_scalar, BassKernelResults)."""
    import time

    from concourse.bass_utils import run_bass_kernel_spmd

    nt, in_maps = _shard_host(a, b, labels)
    nc = _get_program(nt)
    last_err = None
    for attempt in range(3):
        try:
            res = run_bass_kernel_spmd(
                nc, in_maps, list(range(N_CORES)), **run_kwargs
            )
            break
        except Exception as e:  # transient NRT_EXEC_UNIT_UNRECOVERABLE flakes
            last_err = e
            time.sleep(2.0)
    else:
        raise last_err
    partials = np.stack([r["out"][0] for r in res.results])  # [8, 2]
    total = np.float32(partials[:, 0].astype(np.float64).sum())
    cnt = max(int(round(float(partials[:, 1].sum()))), 1)
    value = np.asarray(np.float32(total) / np.float32(cnt))
    return value, res


def kernel(a, b, labels):
    value, _ = run_sharded(a, b, labels)
    return value


# revision 25
# speedup vs baseline: 1.0765x; 1.0179x over previous
"""Trainium2 Bass kernel for CosineSimilarityLoss.

Reference math (see problem):
    x1 = a[labels]; x2 = b[labels]          # gather rows, [N, D]
    ip = sum(x1*x2, -1); w1 = ||x1||; w2 = ||x2||
    cos = ip / max(w1*w2, 1e-8)
    mask = cos >= 0.1
    out = sum(cos[mask]) / max(count(mask), 1)

Sharding: rows of a/b are sharded across 8 cores (12500 rows each). The
host partitions `labels` by owning shard; each core gathers only its local
rows via indirect DMA, computes the masked partial sum and count, and the
host combines the 8 (sum, count) pairs.

Per-core device kernel:
  - inputs:  ab  [12500, 1024] f32 (concat a_shard | b_shard: one gather
             descriptor fetches both rows)
             idx [128, NT] i32 (grid (p,c) holds label slot c*128+p; pads
             point at row 0)
             w   [128, NT] f32 (1.0 real label, 0.0 pad)
  - output:  out [1, 2] f32 = (masked partial sum, partial count)

Engine split per super-chunk of up to 4 label-columns:
  - GpSimd: one indirect DMA per column (128 rows x 4KB) into slices of a
    shared super-tile (Q7 descriptor emission ~10ns/desc is a hard serial
    floor, ~20us for 2176 rows — same for dma_gather, which also needs a
    5us ucode library load, so plain indirect DMA wins)
  - DVE:    batched product a*b + free-axis reduce -> ip over the super-
    tile; plus |b|^2 for one super-chunk (load balance)
  - ACT:    Square activation with fused accumulation for the norms
"""

import math
import sys

import numpy as np

if "/opt/trn_rl_repo" not in sys.path:
    sys.path.append("/opt/trn_rl_repo")


def _ensure_axon_hooks_stub():
    """concourse's axon trace path imports antenv.axon_hooks, which some
    agent images lack; a BASS_TRACE=1 environment would then crash the run.
    Provide a stub that degrades tracing gracefully."""
    try:
        import antenv.axon_hooks  # noqa: F401
        return
    except Exception:
        pass
    try:
        import types

        import antenv

        mod = types.ModuleType("antenv.axon_hooks")
        mod.get_axon_ntff_profile_hook = lambda: None
        mod.set_axon_ntff_profile_hook = lambda h: None
        antenv.axon_hooks = mod
        sys.modules["antenv.axon_hooks"] = mod
    except Exception:
        pass


_ensure_axon_hooks_stub()

V = 100000
D = 512
N_CORES = 8
R = V // N_CORES
P = 128
EPS = 1e-8
MIN_THRESH = 0.1
TG = 4  # label-columns per super-chunk

_CACHE: dict = {}


def _super_chunks(nt: int):
    """Column ranges per super-chunk; first one small to start the compute
    pipeline early."""
    out = []
    c = 0
    first = True
    while c < nt:
        tcs = min(1 if first else TG, nt - c)
        out.append((c, tcs))
        c += tcs
        first = False
    return out


def _build_program(nt: int, rows: int = R, d: int = D):
    import concourse.bacc as bacc
    import concourse.bass as bass
    import concourse.mybir as mybir
    import concourse.tile as tile

    f32 = mybir.dt.float32
    Alu = mybir.AluOpType
    Act = mybir.ActivationFunctionType

    nc = bacc.Bacc(
        "TRN2",
        target_bir_lowering=False,
        debug=False,
        enable_asserts=False,
        num_devices=N_CORES,
    )
    ab = nc.dram_tensor("ab", [rows, 2 * d], f32, kind="ExternalInput").ap()
    idx = nc.dram_tensor("idx", [P, nt], mybir.dt.int32, kind="ExternalInput").ap()
    wv = nc.dram_tensor("w", [P, nt], f32, kind="ExternalInput").ap()
    out = nc.dram_tensor("out", [1, 2], f32, kind="ExternalOutput").ap()

    with tile.TileContext(nc) as tc:
        with (
            tc.tile_pool(name="persist", bufs=1) as persist,
            tc.tile_pool(name="gather", bufs=6) as gpool,
            tc.tile_pool(name="scrv", bufs=2) as scrv,
            tc.tile_pool(name="dumm", bufs=1) as dummp,
            tc.tile_pool(name="tail", bufs=1) as tailp,
            tc.tile_pool(name="psum", bufs=1, space="PSUM") as psump,
        ):
            idx_sb = persist.tile([P, nt], mybir.dt.int32)
            w_sb = persist.tile([P, nt], f32)
            ip_sb = persist.tile([P, nt], f32)
            n1_sb = persist.tile([P, nt], f32)
            n2_sb = persist.tile([P, nt], f32)
            # split the index load so the first gather only waits on col 0
            nc.sync.dma_start(out=idx_sb[:, 0:1], in_=idx[:, 0:1])
            if nt > 1:
                nc.sync.dma_start(out=idx_sb[:, 1:nt], in_=idx[:, 1:nt])
            nc.sync.dma_start(out=w_sb[:], in_=wv)

            # write-only sink for ACT Square ops (their real output is the
            # fused accumulator); same-engine program order makes reuse safe
            act_dummy = dummp.tile([P, 1], f32)
            # pre-warm the Sqrt activation table off the critical path (the
            # first use would otherwise pay a ~1.3us inline table load +
            # engine drain right inside the tail chain)
            warm = dummp.tile([P, 1], f32)
            nc.vector.memset(warm[:], 1.0)
            nc.scalar.activation(act_dummy[:], warm[:], Act.Sqrt)

            for sc_i, (c0, tcs) in enumerate(_super_chunks(nt)):
                g = gpool.tile([P, TG, 2 * d], f32, tag="g")
                for t in range(tcs):
                    nc.gpsimd.indirect_dma_start(
                        out=g[:, t, :],
                        out_offset=None,
                        in_=ab,
                        in_offset=bass.IndirectOffsetOnAxis(
                            ap=idx_sb[:, c0 + t : c0 + t + 1], axis=0
                        ),
                    )
                av = g[:, 0:tcs, 0:d]
                bv = g[:, 0:tcs, d : 2 * d]
                # ip = reduce(a*b), batched over tcs columns (fused DVE accum
                # ops crash/fail to compile on this stack, so two ops; Pool
                # products serialize into the tail after emission — net loss)
                so = scrv.tile([P, TG, d], f32, tag="scrv")
                nc.vector.tensor_tensor(
                    out=so[:, 0:tcs, :], in0=av, in1=bv, op=Alu.mult
                )
                nc.vector.tensor_reduce(
                    ip_sb[:, c0 : c0 + tcs],
                    so[:, 0:tcs, :],
                    axis=mybir.AxisListType.X,
                    op=Alu.add,
                )
                # norms: ACT Square+accum per column, except one super-chunk
                # of n2 on DVE to balance engine load (ACT ~1.0us/col vs DVE
                # ~1.12us/col for a square+reduce)
                n2_dve_cols = tcs if sc_i == 1 else 0
                if n2_dve_cols:
                    so2 = scrv.tile([P, TG, d], f32, tag="scrv")
                    bvd = g[:, 0:n2_dve_cols, d : 2 * d]
                    nc.vector.tensor_tensor(
                        out=so2[:, 0:n2_dve_cols, :], in0=bvd, in1=bvd, op=Alu.mult
                    )
                    nc.vector.tensor_reduce(
                        n2_sb[:, c0 : c0 + n2_dve_cols],
                        so2[:, 0:n2_dve_cols, :],
                        axis=mybir.AxisListType.X,
                        op=Alu.add,
                    )
                for t in range(tcs):
                    c = c0 + t
                    nc.scalar.activation(
                        act_dummy[:].broadcast_to([P, d]),
                        g[:, t, 0:d],
                        Act.Square,
                        accum_out=n1_sb[:, c : c + 1],
                    )
                    if t >= n2_dve_cols:
                        nc.scalar.activation(
                            act_dummy[:].broadcast_to([P, d]),
                            g[:, t, d : 2 * d],
                            Act.Square,
                            accum_out=n2_sb[:, c : c + 1],
                        )

            # tail: cos = ip / max(sqrt(n1*n2), eps); masked sum + count
            nn = tailp.tile([P, nt], f32)
            nc.vector.tensor_tensor(out=nn[:], in0=n1_sb[:], in1=n2_sb[:], op=Alu.mult)
            den0 = tailp.tile([P, nt], f32)
            nc.scalar.activation(den0[:], nn[:], Act.Sqrt)
            den = tailp.tile([P, nt], f32)
            nc.vector.tensor_scalar(
                out=den[:], in0=den0[:], scalar1=EPS, scalar2=None, op0=Alu.max
            )
            rec = tailp.tile([P, nt], f32)
            nc.vector.reciprocal(rec[:], den[:])
            cosv = tailp.tile([P, nt], f32)
            nc.vector.tensor_tensor(out=cosv[:], in0=ip_sb[:], in1=rec[:], op=Alu.mult)
            mk = tailp.tile([P, nt], f32)
            nc.vector.tensor_scalar(
                out=mk[:], in0=cosv[:], scalar1=MIN_THRESH, scalar2=None, op0=Alu.is_ge
            )
            mw = tailp.tile([P, nt], f32)
            nc.vector.tensor_tensor(out=mw[:], in0=mk[:], in1=w_sb[:], op=Alu.mult)
            mc = tailp.tile([P, nt], f32)
            nc.vector.tensor_tensor(out=mc[:], in0=cosv[:], in1=mw[:], op=Alu.mult)

            sc = tailp.tile([P, 2], f32)
            nc.vector.tensor_reduce(
                sc[:, 0:1], mc[:], axis=mybir.AxisListType.X, op=Alu.add
            )
            nc.vector.tensor_reduce(
                sc[:, 1:2], mw[:], axis=mybir.AxisListType.X, op=Alu.add
            )
            ones = tailp.tile([P, 1], f32)
            nc.vector.memset(ones[:], 1.0)
            ps = psump.tile([1, 2], f32, space="PSUM")
            nc.tensor.matmul(ps[:], lhsT=ones[:], rhs=sc[:], start=True, stop=True)
            osb = tailp.tile([1, 2], f32)
            nc.vector.tensor_copy(out=osb[:], in_=ps[:])
            nc.sync.dma_start(out=out, in_=osb[:])

    nc.compile()
    return nc


def _get_program(nt: int):
    key = ("prog", nt)
    if key not in _CACHE:
        _CACHE[key] = _build_program(nt)
    return _CACHE[key]


def _shard_host(a, b, labels):
    """Partition labels by owning row-shard; build per-core inputs."""
    a = np.ascontiguousarray(np.asarray(a, dtype=np.float32))
    b = np.ascontiguousarray(np.asarray(b, dtype=np.float32))
    lab = np.asarray(labels).astype(np.int64).ravel()

    # dedupe duplicate labels per shard: gather each distinct row once and
    # weight its (identical) cosine by the multiplicity — same value and
    # count as the reference, ~8-12% less DMA/compute
    locs = []
    for dcore in range(N_CORES):
        lo = dcore * R
        sel = lab[(lab >= lo) & (lab < lo + R)] - lo
        uniq, cnts = np.unique(sel, return_counts=True)
        locs.append((uniq.astype(np.int32), cnts.astype(np.float32)))
    kmax = max(len(u) for u, _ in locs)
    nt = max(1, math.ceil(kmax / P))
    kpad = nt * P

    in_maps = []
    for dcore in range(N_CORES):
        lo = dcore * R
        uniq, cnts = locs[dcore]
        flat = np.zeros(kpad, dtype=np.int32)
        flat[: len(uniq)] = uniq
        w_flat = np.zeros(kpad, dtype=np.float32)
        w_flat[: len(uniq)] = cnts
        # grid position (p, c) holds flat slot c*128+p
        idx2d = np.ascontiguousarray(flat.reshape(nt, P).T)
        w2d = np.ascontiguousarray(w_flat.reshape(nt, P).T)
        ab = np.concatenate([a[lo : lo + R], b[lo : lo + R]], axis=1)
        in_maps.append(
            {"ab": np.ascontiguousarray(ab), "idx": idx2d, "w": w2d}
        )
    return nt, in_maps


def run_sharded(a, b, labels, **run_kwargs):
    """Shard, run on 8 cores, return (result_scalar, BassKernelResults)."""
    import time

    from concourse.bass_utils import run_bass_kernel_spmd

    nt, in_maps = _shard_host(a, b, labels)
    nc = _get_program(nt)
    last_err = None
    for attempt in range(3):
        try:
            res = run_bass_kernel_spmd(
                nc, in_maps, list(range(N_CORES)), **run_kwargs
            )
            break
        except Exception as e:  # transient NRT_EXEC_UNIT_UNRECOVERABLE flakes
            last_err = e
            time.sleep(2.0)
    else:
        raise last_err
    partials = np.stack([r["out"][0] for r in res.results])  # [8, 2]
    total = np.float32(partials[:, 0].astype(np.float64).sum())
    cnt = max(int(round(float(partials[:, 1].sum()))), 1)
    value = np.asarray(np.float32(total) / np.float32(cnt))
    return value, res


def kernel(a, b, labels):
    value, _ = run_sharded(a, b, labels)
    return value
